# revision 33
# baseline (speedup 1.0000x reference)
"""EAST-style loss (weighted BCE score + smoothed-L1 geometry) on 8 trn2 cores.

Strategy: pure data parallel over batch m=128 -> 16 per core. Host packs each
core's shard into reduced precision: geometry in fp8-e4m3 (quantization shifts
the geometry loss ~0.4%, which is ~1e-5 of the total loss), score in fp16
(~1e-4 noise; fp8 would wreck ln(yp) near the clip bounds). Rel-err budget is
2e-2. Each core streams 5MB through SBUF:
  xg{i} [128, 2*FGS[i]] fp8: geometry pair-tiles, cols [0:f]=yt, [f:2f]=yp
  xa/xnb [128, FA] fp8:      one geometry chunk shipped as yt and -yp
  xs    [128, 2*2048] fp16:  score, cols 0:2048 = yt, 2048: = yp

The Vector engine is the end-to-end bottleneck, so geometry is split:
- FGS tiles: ONE fused custom-DVE op per pair-tile (registered via the
  documented dve_ops extension point): with d = a-b, c = clamp(d,-1,1),
    huber(d) = d*c - 0.5*c^2     (= 0.5 d^2 inside, |d|-0.5 outside)
  summed across the free dim by the op's accumulator - one DVE pass/element.
- FA chunk: offloaded to the otherwise-idle ACT engine. The DMA engine
  itself computes d = yt + (-yp) inline (SWDGE CCE-accumulate; host ships
  -yp), then ACT does Abs -> Square-accum / Relu(-1) -> Square-accum:
    sum huber = 0.5*sum(|d|^2) - 0.5*sum(relu(|d|-1)^2)

Score: clamp yp below 1.0 (fp16 rounds 1-1e-4 up to 1.0 -> ln(0)), ln/ln(1-.)
on ACT with accum, yt*ln products on DVE with accum. add_dep_helper pins
shape the static per-engine programs (engines execute their program in order,
so a mis-slotted op head-of-line blocks): score products interleave between
hubers on DVE, lns run before the chunk chain on ACT. Final scalar combine
happens on host in float64 (stats are tiny: [128, 10]).
"""

import sys

sys.path.insert(0, "/opt/trn_rl_repo")

import numpy as np

import concourse.bacc as bacc
import concourse.mybir as mybir
from concourse.bass_utils import run_bass_kernel_spmd
from concourse.tile import TileContext

N_CORES = 8
M, H, W = 128, 128, 128
GC = 8  # geometry channels
M_PER = M // N_CORES  # 16

P = 128
# geometry pair-tile half-widths; (sum(FGS) + FA) * 128 = 2,097,152 elems per
# core per tensor. FGS tiles go to the custom-DVE huber; the FA chunk goes to
# the ACT engine (see module docstring).
FGS = [4096, 4096, 4096, 2048]
FA = 2048
N_GT = len(FGS)
FG_OFF = [0]
for _f in FGS:
    FG_OFF.append(FG_OFF[-1] + _f)
FS = 2048  # score free-dim per half (fp16)

# fp16-representable clamp just below 1.0 so ln(1-yp) stays finite
YP_MAX = 0.99951171875

# stats columns (single fp32 [P, N_GT+6] tensor):
#   [0:N_GT]  = sum huber(d) per geometry tile   (custom DVE accum)
#   [N_GT]    = sum(ln(1-yp))                    (ACT accum)
#   [N_GT+1]  = sum(yt_s)                        (ACT accum)
#   [N_GT+2]  = sum(yt_s * ln(yp))               (DVE accum)
#   [N_GT+3]  = sum(yt_s * ln(1-yp))             (DVE accum)
#   [N_GT+4]  = sum(|d|^2)   for the ACT chunk   (ACT accum)
#   [N_GT+5]  = sum(relu(|d|-1)^2) for the chunk (ACT accum)
NS = N_GT + 6

F16 = mybir.dt.float16
F8 = mybir.dt.float8e4
F32 = mybir.dt.float32

_CACHED_NC = None
_HUBER_OP = None


def _register_huber_op():
    """Register the fused huber+accumulate custom-DVE op (idempotent).

    Uses the documented dve_ops extension point (04-custom-dve-api.md): the
    op's uop program is written into the per-NEFF DVE table at compile time.
    """
    global _HUBER_OP
    if _HUBER_OP is not None:
        return _HUBER_OP
    from concourse import dve_ops as DO
    from concourse.dve_spec import (
        AluOp, C2, One, Spec, Src0, Src1, Zero, lower, maxx, minn, sq,
    )
    from concourse.dve_table_gen import dve_ver_for
    from concourse.dve_uop import DveOpSpec

    name = "HUBER_ACC_ANT"
    if name in DO._SUB_OPCODE_FOR_NAME:
        _HUBER_OP = next(op for op in DO.OPS if op.name == name)
        return _HUBER_OP
    d = Src0 - Src1
    c = maxx(minn(d, One), Zero - One)
    spec = Spec(body=d * c - sq(c) * C2, accum=AluOp.ADD)  # imm2 = 0.5
    ver = dve_ver_for("TRN2")
    row = max(DO._SUB_OPCODE_FOR_NAME.values()) + 1
    sha = DveOpSpec(
        name=name, opcode=row, uops=lower(spec, ver=ver), rd1_en=True
    ).sha(ver)
    op = DO.DveOp(name, spec, subdim=False, uops_sha={ver: sha})
    DO.OPS.append(op)
    DO._SUB_OPCODE_FOR_NAME[name] = row
    DO.CUSTOM_DVE_SPECS[name] = spec
    _HUBER_OP = op
    return op


def _build_nc():
    huber_op = _register_huber_op()
    nc = bacc.Bacc("TRN2", target_bir_lowering=False)
    # one contiguous DRAM block per tile (strided column-slices of a single
    # big tensor measured ~10% slower HBM streaming)
    xg_d = [
        nc.dram_tensor(f"xg{i}", [P, 2 * FGS[i]], F8, kind="ExternalInput")
        for i in range(N_GT)
    ]
    xa_d = nc.dram_tensor("xa", [P, FA], F8, kind="ExternalInput")  # yt chunk
    xnb_d = nc.dram_tensor("xnb", [P, FA], F8, kind="ExternalInput")  # -yp
    xs_d = nc.dram_tensor("xs", [P, 2 * FS], F16, kind="ExternalInput")
    st_d = nc.dram_tensor("st", [P, NS], F32, kind="ExternalOutput")

    AF = mybir.ActivationFunctionType
    OP = mybir.AluOpType

    from concourse.tile_rust import add_dep_helper

    with TileContext(nc) as tc:
        with (
            tc.tile_pool(name="stats", bufs=1) as spool,
            tc.tile_pool(name="io", bufs=1) as iopool,
            tc.tile_pool(name="score", bufs=1) as scpool,
            tc.tile_pool(name="work", bufs=3) as wpool,
        ):
            st = spool.tile([P, NS], F32)
            cm1 = spool.tile([P, 1], F32)  # Relu bias -1.0 for the ACT chunk
            nc.vector.memset(cm1[:], -1.0)

            # ---------------- input DMAs (all tiles SBUF-resident) ----------
            # Queue order = arrival order: xg0 first so the DVE starts ~3us
            # earlier; then the ACT chunk (its d must exist before ACT's
            # mid-schedule slot); score next (gates the clamp+ln chain);
            # remaining geometry big-to-small.
            xg = [None] * N_GT
            t = iopool.tile([P, 2 * FGS[0]], F8, tag="xg0")
            nc.sync.dma_start(out=t[:], in_=xg_d[0][:])
            xg[0] = t
            da = iopool.tile([P, FA], F8, tag="da")
            nc.sync.dma_start(out=da[:], in_=xa_d[:])
            # d = yt + (-yp), computed inline by the DMA engine (SWDGE CCE)
            nc.gpsimd.dma_start(out=da[:], in_=xnb_d[:], accum_op=OP.add)
            xs = scpool.tile([P, 2 * FS], F16)
            nc.sync.dma_start(out=xs[:], in_=xs_d[:])
            for i in range(1, N_GT):
                t = iopool.tile([P, 2 * FGS[i]], F8, tag=f"xg{i}")
                nc.sync.dma_start(out=t[:], in_=xg_d[i][:])
                xg[i] = t

            yt = xs[:, 0:FS]
            yp = xs[:, FS : 2 * FS]

            # ---------------- score part ------------------------------------
            ypc = scpool.tile([P, FS], F16)
            i_clamp = nc.vector.tensor_scalar(
                out=ypc[:], in0=yp, scalar1=YP_MAX, scalar2=None, op0=OP.min
            )
            lnp = scpool.tile([P, FS], F16)
            nc.scalar.activation(lnp[:], ypc[:], AF.Ln)
            ln1m = scpool.tile([P, FS], F16)
            i_ln1m = nc.scalar.activation(
                ln1m[:], ypc[:], AF.Ln, scale=-1.0, bias=1.0,
                accum_out=st[:, N_GT : N_GT + 1],
            )
            scr = scpool.tile([P, FS], F16, tag="scr")
            i_stt1 = nc.vector.scalar_tensor_tensor(
                out=scr[:], in0=yt, scalar=1.0, in1=lnp[:],
                op0=OP.mult, op1=OP.mult,
                accum_out=st[:, N_GT + 2 : N_GT + 3],
            )
            scr2 = scpool.tile([P, FS], F16, tag="scr")
            i_stt2 = nc.vector.scalar_tensor_tensor(
                out=scr2[:], in0=yt, scalar=1.0, in1=ln1m[:],
                op0=OP.mult, op1=OP.mult,
                accum_out=st[:, N_GT + 3 : N_GT + 4],
            )

            # ------------- ACT-offload huber chunk (d precomputed by DMA) ---
            ua = scpool.tile([P, FA], F16)
            i_abs = nc.scalar.activation(ua[:], da[:], AF.Abs)
            # lns run first on ACT: they gate the DVE score products
            add_dep_helper(
                i_abs.ins, i_ln1m.ins, sync=False,
                reason="order score lns before ACT huber chunk",
            )
            squ = scpool.tile([P, FA], F16)
            nc.scalar.activation(
                squ[:], ua[:], AF.Square, accum_out=st[:, N_GT + 4 : N_GT + 5]
            )
            ra = scpool.tile([P, FA], F16)
            nc.scalar.activation(ra[:], ua[:], AF.Relu, bias=cm1[:])
            sqr = scpool.tile([P, FA], F16)
            i_sqr = nc.scalar.activation(
                sqr[:], ra[:], AF.Square, accum_out=st[:, N_GT + 5 : N_GT + 6]
            )
            # the sum(yt) copy has no consumers -- park it at the very end
            syt = scpool.tile([P, FS], F16)
            i_copy = nc.scalar.activation(
                syt[:], yt, AF.Copy, accum_out=st[:, N_GT + 1 : N_GT + 2]
            )
            add_dep_helper(
                i_copy.ins, i_sqr.ins, sync=False,
                reason="order ACT huber chunk before the sum(yt) copy",
            )

            # ---------------- geometry part: 1 fused DVE op per pair-tile ---
            # Target DVE order: [h0, clamp, h1, stt1, h2, stt2, h3]
            for i in range(N_GT):
                f = FGS[i]
                h = wpool.tile([P, f], F16, tag="h")
                i_h = nc.vector._custom_dve(
                    huber_op,
                    out=h[:],
                    in0=xg[i][:, 0:f],
                    in1=xg[i][:, f : 2 * f],
                    s0=0.0, s1=0.0, imm2=0.5,
                    accum_out=st[:, i : i + 1],
                )
                if i == 0:
                    add_dep_helper(
                        i_clamp.ins, i_h.ins, sync=False,
                        reason="huber 0 first on DVE (its data lands first)",
                    )
                elif i == 2:
                    add_dep_helper(
                        i_h.ins, i_stt1.ins, sync=False,
                        reason="order first score product before huber 2",
                    )
                elif i == 3:
                    add_dep_helper(
                        i_h.ins, i_stt2.ins, sync=False,
                        reason="order second score product before huber 3",
                    )

            nc.sync.dma_start(out=st_d[:], in_=st[:])
    nc.finalize()
    return nc


def _get_nc():
    global _CACHED_NC
    if _CACHED_NC is None:
        _CACHED_NC = _build_nc()
    return _CACHED_NC


def _make_in_maps(Y_true_score, Y_pred_score, Y_true_geometry, Y_pred_geometry):
    FG_TOT = FG_OFF[-1] + FA  # 16384 geometry elems per partition per tensor
    yts = np.asarray(Y_true_score, dtype=np.float32).reshape(N_CORES, P, FS)
    yps = np.asarray(Y_pred_score, dtype=np.float32).reshape(N_CORES, P, FS)
    ytg = np.asarray(Y_true_geometry, dtype=np.float32).reshape(N_CORES, P, FG_TOT)
    ypg = np.asarray(Y_pred_geometry, dtype=np.float32).reshape(N_CORES, P, FG_TOT)

    xs = np.empty((N_CORES, P, 2 * FS), dtype=np.float16)
    xs[:, :, 0:FS] = yts
    xs[:, :, FS:] = yps
    np8 = mybir.dt.np(F8)
    xgs = []
    for i in range(N_GT):
        o, f = FG_OFF[i], FGS[i]
        xg = np.empty((N_CORES, P, 2 * f), dtype=np8)
        xg[:, :, 0:f] = ytg[:, :, o : o + f]
        xg[:, :, f:] = ypg[:, :, o : o + f]
        xgs.append(xg)
    oa = FG_OFF[-1]
    xa = ytg[:, :, oa:].astype(np8)
    xnb = (-ypg[:, :, oa:]).astype(np8)

    return [
        {
            "xs": xs[k],
            "xa": xa[k],
            "xnb": xnb[k],
            **{f"xg{i}": xgs[i][k] for i in range(N_GT)},
        }
        for k in range(N_CORES)
    ]


def _combine(results):
    """results: list of per-core dicts with st [P, NS] fp32."""
    huber_sum = 0.0
    ln1m_sum = 0.0
    yt_sum = 0.0
    t1_sum = 0.0  # sum yt*ln(yp)
    t2_sum = 0.0  # sum yt*ln(1-yp)
    for r in results:
        s = np.asarray(r["st"], dtype=np.float64)
        huber_sum += s[:, 0:N_GT].sum()
        # ACT chunk: sum huber = 0.5*sum(|d|^2) - 0.5*sum(relu(|d|-1)^2)
        huber_sum += 0.5 * (s[:, N_GT + 4].sum() - s[:, N_GT + 5].sum())
        ln1m_sum += s[:, N_GT].sum()
        yt_sum += s[:, N_GT + 1].sum()
        t1_sum += s[:, N_GT + 2].sum()
        t2_sum += s[:, N_GT + 3].sum()

    size = float(M * 1 * H * W)
    beta = 1.0 - yt_sum / size
    A = t1_sum  # sum(yt * ln yp)
    B = ln1m_sum - t2_sum  # sum((1-yt) * ln(1-yp))
    loss_score = (-beta * A - (1.0 - beta) * B) / M

    n_pix = M * H * W
    loss_geom = huber_sum / GC / n_pix  # LAMBDA_GEOMETRY = 1.0

    return np.array(loss_score + loss_geom, dtype=np.float32)


def kernel(Y_true_score, Y_pred_score, Y_true_geometry, Y_pred_geometry, **_kw):
    nc = _get_nc()
    in_maps = _make_in_maps(
        Y_true_score, Y_pred_score, Y_true_geometry, Y_pred_geometry
    )
    res = run_bass_kernel_spmd(nc, in_maps, core_ids=list(range(N_CORES)))
    return _combine(res.results)


# revision 34
# speedup vs baseline: 1.0508x; 1.0508x over previous
"""EAST-style loss (weighted BCE score + smoothed-L1 geometry) on 8 trn2 cores.

Strategy: pure data parallel over batch m=128 -> 16 per core. Host packs each
core's shard into fp16 (halves HBM traffic; rel-err budget 2e-2 vs ~1e-4 fp16
quantization noise). Each core streams 9MB through SBUF (memory-bound):
  xg [4,128,8192]: geometry pair-tiles, cols 0:4096 = yt chunk, 4096: = yp
  xs [128,4096]:   score,          cols 0:2048 = yt_s,  2048: = yp_s

Geometry uses ONE fused custom-DVE op per pair-tile (registered via the
documented dve_ops extension point): with d = a-b, c = clamp(d,-1,1),
  huber(d) = d*c - 0.5*c^2       (= 0.5 d^2 inside, |d|-0.5 outside)
summed across the free dim by the op's accumulator -> zero ACT work for
geometry, one DVE pass per element. Score: clamp yp below 1.0 (fp16 rounds
1-1e-4 up to 1.0 -> ln(0)), ln/ln(1-.) on ACT with accum, yt*ln products on
DVE with accum. Final scalar combine happens on host in float64 (stats are
tiny: [128, 8]).
"""

import sys

sys.path.insert(0, "/opt/trn_rl_repo")

import numpy as np

import concourse.bacc as bacc
import concourse.mybir as mybir
from concourse.bass_utils import run_bass_kernel_spmd
from concourse.tile import TileContext

N_CORES = 8
M, H, W = 128, 128, 128
GC = 8  # geometry channels
M_PER = M // N_CORES  # 16

P = 128
# geometry pair-tile half-widths; sum(FGS) * 128 = 2,097,152 elems per core
# per tensor. Ramp small->big: the DVE (the end-to-end bottleneck) can start
# on a 0.25MB first tile ~2.5us before a 1MB one would land, and later
# arrivals stay ahead of it. Small final tile shortens the serial tail.
FGS = [1024, 2048, 4096, 4096, 4096, 1024]
N_GT = len(FGS)
FG_OFF = [0]
for _f in FGS:
    FG_OFF.append(FG_OFF[-1] + _f)
FS = 2048  # score free-dim per half (fp16)

# fp16-representable clamp just below 1.0 so ln(1-yp) stays finite
YP_MAX = 0.99951171875

# stats columns (single fp32 [P, N_GT+4] tensor):
#   [0:N_GT]  = sum huber(d) per geometry tile   (custom DVE accum)
#   [N_GT]    = sum(ln(1-yp))                    (ACT accum)
#   [N_GT+1]  = sum(yt_s)                        (ACT accum)
#   [N_GT+2]  = sum(yt_s * ln(yp))               (DVE accum)
#   [N_GT+3]  = sum(yt_s * ln(1-yp))             (DVE accum)
NS = N_GT + 4

F16 = mybir.dt.float16
F8 = mybir.dt.float8e4
F32 = mybir.dt.float32

_CACHED_NC = None
_HUBER_OP = None


def _register_huber_op():
    """Register the fused huber+accumulate custom-DVE op (idempotent).

    Uses the documented dve_ops extension point (04-custom-dve-api.md): the
    op's uop program is written into the per-NEFF DVE table at compile time.
    """
    global _HUBER_OP
    if _HUBER_OP is not None:
        return _HUBER_OP
    from concourse import dve_ops as DO
    from concourse.dve_spec import (
        AluOp, C2, One, Spec, Src0, Src1, Zero, lower, maxx, minn, sq,
    )
    from concourse.dve_table_gen import dve_ver_for
    from concourse.dve_uop import DveOpSpec

    name = "HUBER_ACC_ANT"
    if name in DO._SUB_OPCODE_FOR_NAME:
        _HUBER_OP = next(op for op in DO.OPS if op.name == name)
        return _HUBER_OP
    d = Src0 - Src1
    c = maxx(minn(d, One), Zero - One)
    spec = Spec(body=d * c - sq(c) * C2, accum=AluOp.ADD)  # imm2 = 0.5
    ver = dve_ver_for("TRN2")
    row = max(DO._SUB_OPCODE_FOR_NAME.values()) + 1
    sha = DveOpSpec(
        name=name, opcode=row, uops=lower(spec, ver=ver), rd1_en=True
    ).sha(ver)
    op = DO.DveOp(name, spec, subdim=False, uops_sha={ver: sha})
    DO.OPS.append(op)
    DO._SUB_OPCODE_FOR_NAME[name] = row
    DO.CUSTOM_DVE_SPECS[name] = spec
    _HUBER_OP = op
    return op


def _build_nc():
    huber_op = _register_huber_op()
    nc = bacc.Bacc("TRN2", target_bir_lowering=False)
    # one contiguous DRAM block per graded tile (strided column-slices of a
    # single big tensor measured ~10% slower HBM streaming)
    xg_d = [
        nc.dram_tensor(f"xg{i}", [P, 2 * FGS[i]], F8, kind="ExternalInput")
        for i in range(N_GT)
    ]
    xs_d = nc.dram_tensor("xs", [P, 2 * FS], F16, kind="ExternalInput")
    st_d = nc.dram_tensor("st", [P, NS], F32, kind="ExternalOutput")

    AF = mybir.ActivationFunctionType
    OP = mybir.AluOpType

    with TileContext(nc) as tc:
        with (
            tc.tile_pool(name="stats", bufs=1) as spool,
            tc.tile_pool(name="io", bufs=1) as iopool,
            tc.tile_pool(name="score", bufs=1) as scpool,
            tc.tile_pool(name="work", bufs=3) as wpool,
        ):
            st = spool.tile([P, NS], F32)

            # ---------------- input DMAs (all tiles SBUF-resident) ----------
            # Queue order = arrival order: geometry ramp first so the DVE
            # starts as early as possible; score mid-stream (its clamp ->
            # ln -> product chain fits into DVE slots after huber 3).
            xg = [None] * N_GT
            for i in range(3):
                t = iopool.tile([P, 2 * FGS[i]], F8, tag=f"xg{i}")
                nc.sync.dma_start(out=t[:], in_=xg_d[i][:])
                xg[i] = t
            xs = scpool.tile([P, 2 * FS], F16)
            nc.sync.dma_start(out=xs[:], in_=xs_d[:])
            for i in range(3, N_GT):
                t = iopool.tile([P, 2 * FGS[i]], F8, tag=f"xg{i}")
                nc.sync.dma_start(out=t[:], in_=xg_d[i][:])
                xg[i] = t

            yt = xs[:, 0:FS]
            yp = xs[:, FS : 2 * FS]

            # ---------------- score part ------------------------------------
            ypc = scpool.tile([P, FS], F16)
            i_clamp = nc.vector.tensor_scalar(
                out=ypc[:], in0=yp, scalar1=YP_MAX, scalar2=None, op0=OP.min
            )
            from concourse.tile_rust import add_dep_helper

            lnp = scpool.tile([P, FS], F16)
            nc.scalar.activation(lnp[:], ypc[:], AF.Ln)
            ln1m = scpool.tile([P, FS], F16)
            i_ln1m = nc.scalar.activation(
                ln1m[:], ypc[:], AF.Ln, scale=-1.0, bias=1.0,
                accum_out=st[:, N_GT : N_GT + 1],
            )
            syt = scpool.tile([P, FS], F16)
            i_copy = nc.scalar.activation(
                syt[:], yt, AF.Copy, accum_out=st[:, N_GT + 1 : N_GT + 2]
            )
            # keep ACT's static order ln -> ln(1-.) -> copy: the copy is not
            # on the critical chain, but scheduled first it delays both lns
            # (and with them the DVE products) by ~3.5us.
            add_dep_helper(
                i_copy.ins, i_ln1m.ins, sync=False,
                reason="order score lns before the sum(yt) copy",
            )
            scr = scpool.tile([P, FS], F16, tag="scr")
            i_stt1 = nc.vector.scalar_tensor_tensor(
                out=scr[:], in0=yt, scalar=1.0, in1=lnp[:],
                op0=OP.mult, op1=OP.mult,
                accum_out=st[:, N_GT + 2 : N_GT + 3],
            )
            scr2 = scpool.tile([P, FS], F16, tag="scr")
            i_stt2 = nc.vector.scalar_tensor_tensor(
                out=scr2[:], in0=yt, scalar=1.0, in1=ln1m[:],
                op0=OP.mult, op1=OP.mult,
                accum_out=st[:, N_GT + 3 : N_GT + 4],
            )

            # ---------------- geometry part: 1 fused DVE op per pair-tile ---
            # Pin DVE static order [h0, h1, h2, clamp, h3, stt1, h4, stt2,
            # h5]: matches the arrival ramp, score products fill the DVE
            # while big tiles stream in, and nothing trails the last byte
            # (engine programs are static; a mis-slotted op head-of-line
            # blocks the engine).
            hs = []
            for i in range(N_GT):
                f = FGS[i]
                h = wpool.tile([P, f], F16, tag="h")
                i_h = nc.vector._custom_dve(
                    huber_op,
                    out=h[:],
                    in0=xg[i][:, 0:f],
                    in1=xg[i][:, f : 2 * f],
                    s0=0.0, s1=0.0, imm2=0.5,
                    accum_out=st[:, i : i + 1],
                )
                hs.append(i_h)
            add_dep_helper(i_clamp.ins, hs[2].ins, sync=False,
                           reason="clamp after huber 2")
            add_dep_helper(i_stt1.ins, hs[3].ins, sync=False,
                           reason="stt1 after huber 3")
            add_dep_helper(hs[4].ins, i_stt1.ins, sync=False,
                           reason="huber 4 after stt1")
            add_dep_helper(i_stt2.ins, hs[4].ins, sync=False,
                           reason="stt2 after huber 4")
            add_dep_helper(hs[5].ins, i_stt2.ins, sync=False,
                           reason="huber 5 after stt2")

            nc.sync.dma_start(out=st_d[:], in_=st[:])
    nc.finalize()
    return nc


def _get_nc():
    global _CACHED_NC
    if _CACHED_NC is None:
        _CACHED_NC = _build_nc()
    return _CACHED_NC


def _make_in_maps(Y_true_score, Y_pred_score, Y_true_geometry, Y_pred_geometry):
    FG_TOT = FG_OFF[-1]  # 16384 geometry elems per partition per tensor
    yts = np.asarray(Y_true_score, dtype=np.float32).reshape(N_CORES, P, FS)
    yps = np.asarray(Y_pred_score, dtype=np.float32).reshape(N_CORES, P, FS)
    ytg = np.asarray(Y_true_geometry, dtype=np.float32).reshape(N_CORES, P, FG_TOT)
    ypg = np.asarray(Y_pred_geometry, dtype=np.float32).reshape(N_CORES, P, FG_TOT)

    xs = np.empty((N_CORES, P, 2 * FS), dtype=np.float16)
    xs[:, :, 0:FS] = yts
    xs[:, :, FS:] = yps
    np8 = mybir.dt.np(F8)
    xgs = []
    for i in range(N_GT):
        o, f = FG_OFF[i], FGS[i]
        xg = np.empty((N_CORES, P, 2 * f), dtype=np8)
        xg[:, :, 0:f] = ytg[:, :, o : o + f]
        xg[:, :, f:] = ypg[:, :, o : o + f]
        xgs.append(xg)

    return [
        {"xs": xs[k], **{f"xg{i}": xgs[i][k] for i in range(N_GT)}}
        for k in range(N_CORES)
    ]


def _combine(results):
    """results: list of per-core dicts with st [P, NS] fp32."""
    huber_sum = 0.0
    ln1m_sum = 0.0
    yt_sum = 0.0
    t1_sum = 0.0  # sum yt*ln(yp)
    t2_sum = 0.0  # sum yt*ln(1-yp)
    for r in results:
        s = np.asarray(r["st"], dtype=np.float64)
        huber_sum += s[:, 0:N_GT].sum()
        ln1m_sum += s[:, N_GT].sum()
        yt_sum += s[:, N_GT + 1].sum()
        t1_sum += s[:, N_GT + 2].sum()
        t2_sum += s[:, N_GT + 3].sum()

    size = float(M * 1 * H * W)
    beta = 1.0 - yt_sum / size
    A = t1_sum  # sum(yt * ln yp)
    B = ln1m_sum - t2_sum  # sum((1-yt) * ln(1-yp))
    loss_score = (-beta * A - (1.0 - beta) * B) / M

    n_pix = M * H * W
    loss_geom = huber_sum / GC / n_pix  # LAMBDA_GEOMETRY = 1.0

    return np.array(loss_score + loss_geom, dtype=np.float32)


def kernel(Y_true_score, Y_pred_score, Y_true_geometry, Y_pred_geometry, **_kw):
    nc = _get_nc()
    in_maps = _make_in_maps(
        Y_true_score, Y_pred_score, Y_true_geometry, Y_pred_geometry
    )
    res = run_bass_kernel_spmd(nc, in_maps, core_ids=list(range(N_CORES)))
    return _combine(res.results)


# revision 35
# speedup vs baseline: 1.0619x; 1.0106x over previous
"""EAST-style loss (weighted BCE score + smoothed-L1 geometry) on 8 trn2 cores.

Strategy: pure data parallel over batch m=128 -> 16 per core. Host packs each
core's shard into fp16 (halves HBM traffic; rel-err budget 2e-2 vs ~1e-4 fp16
quantization noise). Each core streams 9MB through SBUF (memory-bound):
  xg [4,128,8192]: geometry pair-tiles, cols 0:4096 = yt chunk, 4096: = yp
  xs [128,4096]:   score,          cols 0:2048 = yt_s,  2048: = yp_s

Geometry uses ONE fused custom-DVE op per pair-tile (registered via the
documented dve_ops extension point): with d = a-b, c = clamp(d,-1,1),
  huber(d) = d*c - 0.5*c^2       (= 0.5 d^2 inside, |d|-0.5 outside)
summed across the free dim by the op's accumulator -> zero ACT work for
geometry, one DVE pass per element. Score: clamp yp below 1.0 (fp16 rounds
1-1e-4 up to 1.0 -> ln(0)), ln/ln(1-.) on ACT with accum, yt*ln products on
DVE with accum. Final scalar combine happens on host in float64 (stats are
tiny: [128, 8]).
"""

import sys

sys.path.insert(0, "/opt/trn_rl_repo")

import numpy as np

import concourse.bacc as bacc
import concourse.mybir as mybir
from concourse.bass_utils import run_bass_kernel_spmd
from concourse.tile import TileContext

N_CORES = 8
M, H, W = 128, 128, 128
GC = 8  # geometry channels
M_PER = M // N_CORES  # 16

P = 128
# geometry pair-tile half-widths; sum(FGS) * 128 = 2,097,152 elems per core
# per tensor. Ramp small->big: the DVE (the end-to-end bottleneck) can start
# on a 0.25MB first tile ~2.5us before a 1MB one would land, and later
# arrivals stay ahead of it. Small final tile shortens the serial tail.
FGS = [1536, 2560, 4096, 4096, 3072, 1024]
N_GT = len(FGS)
FG_OFF = [0]
for _f in FGS:
    FG_OFF.append(FG_OFF[-1] + _f)
FS = 2048  # score free-dim per half (fp16)

# fp16-representable clamp just below 1.0 so ln(1-yp) stays finite
YP_MAX = 0.99951171875

# stats columns (single fp32 [P, N_GT+4] tensor):
#   [0:N_GT]  = sum huber(d) per geometry tile   (custom DVE accum)
#   [N_GT]    = sum(ln(1-yp))                    (ACT accum)
#   [N_GT+1]  = sum(yt_s)                        (ACT accum)
#   [N_GT+2]  = sum(yt_s * ln(yp))               (DVE accum)
#   [N_GT+3]  = sum(yt_s * ln(1-yp))             (DVE accum)
NS = N_GT + 4

F16 = mybir.dt.float16
F8 = mybir.dt.float8e4
F32 = mybir.dt.float32

_CACHED_NC = None
_HUBER_OP = None


def _register_huber_op():
    """Register the fused huber+accumulate custom-DVE op (idempotent).

    Uses the documented dve_ops extension point (04-custom-dve-api.md): the
    op's uop program is written into the per-NEFF DVE table at compile time.
    """
    global _HUBER_OP
    if _HUBER_OP is not None:
        return _HUBER_OP
    from concourse import dve_ops as DO
    from concourse.dve_spec import (
        AluOp, C2, One, Spec, Src0, Src1, Zero, lower, maxx, minn, sq,
    )
    from concourse.dve_table_gen import dve_ver_for
    from concourse.dve_uop import DveOpSpec

    name = "HUBER_ACC_ANT"
    if name in DO._SUB_OPCODE_FOR_NAME:
        _HUBER_OP = next(op for op in DO.OPS if op.name == name)
        return _HUBER_OP
    d = Src0 - Src1
    c = maxx(minn(d, One), Zero - One)
    spec = Spec(body=d * c - sq(c) * C2, accum=AluOp.ADD)  # imm2 = 0.5
    ver = dve_ver_for("TRN2")
    row = max(DO._SUB_OPCODE_FOR_NAME.values()) + 1
    sha = DveOpSpec(
        name=name, opcode=row, uops=lower(spec, ver=ver), rd1_en=True
    ).sha(ver)
    op = DO.DveOp(name, spec, subdim=False, uops_sha={ver: sha})
    DO.OPS.append(op)
    DO._SUB_OPCODE_FOR_NAME[name] = row
    DO.CUSTOM_DVE_SPECS[name] = spec
    _HUBER_OP = op
    return op


def _build_nc():
    huber_op = _register_huber_op()
    nc = bacc.Bacc("TRN2", target_bir_lowering=False)
    # one contiguous DRAM block per graded tile (strided column-slices of a
    # single big tensor measured ~10% slower HBM streaming)
    xg_d = [
        nc.dram_tensor(f"xg{i}", [P, 2 * FGS[i]], F8, kind="ExternalInput")
        for i in range(N_GT)
    ]
    xs_d = nc.dram_tensor("xs", [P, 2 * FS], F16, kind="ExternalInput")
    st_d = nc.dram_tensor("st", [P, NS], F32, kind="ExternalOutput")

    AF = mybir.ActivationFunctionType
    OP = mybir.AluOpType

    with TileContext(nc) as tc:
        with (
            tc.tile_pool(name="stats", bufs=1) as spool,
            tc.tile_pool(name="io", bufs=1) as iopool,
            tc.tile_pool(name="score", bufs=1) as scpool,
            tc.tile_pool(name="work", bufs=3) as wpool,
        ):
            st = spool.tile([P, NS], F32)

            # ---------------- input DMAs (all tiles SBUF-resident) ----------
            # Queue order = arrival order: geometry ramp first so the DVE
            # starts as early as possible; score mid-stream (its clamp ->
            # ln -> product chain fits into DVE slots after huber 3).
            xg = [None] * N_GT
            for i in range(3):
                t = iopool.tile([P, 2 * FGS[i]], F8, tag=f"xg{i}")
                nc.sync.dma_start(out=t[:], in_=xg_d[i][:])
                xg[i] = t
            xs = scpool.tile([P, 2 * FS], F16)
            nc.sync.dma_start(out=xs[:], in_=xs_d[:])
            for i in range(3, N_GT):
                t = iopool.tile([P, 2 * FGS[i]], F8, tag=f"xg{i}")
                nc.sync.dma_start(out=t[:], in_=xg_d[i][:])
                xg[i] = t

            yt = xs[:, 0:FS]
            yp = xs[:, FS : 2 * FS]

            # ---------------- score part ------------------------------------
            ypc = scpool.tile([P, FS], F16)
            i_clamp = nc.vector.tensor_scalar(
                out=ypc[:], in0=yp, scalar1=YP_MAX, scalar2=None, op0=OP.min
            )
            from concourse.tile_rust import add_dep_helper

            lnp = scpool.tile([P, FS], F16)
            nc.scalar.activation(lnp[:], ypc[:], AF.Ln)
            ln1m = scpool.tile([P, FS], F16)
            i_ln1m = nc.scalar.activation(
                ln1m[:], ypc[:], AF.Ln, scale=-1.0, bias=1.0,
                accum_out=st[:, N_GT : N_GT + 1],
            )
            syt = scpool.tile([P, FS], F16)
            i_copy = nc.scalar.activation(
                syt[:], yt, AF.Copy, accum_out=st[:, N_GT + 1 : N_GT + 2]
            )
            # keep ACT's static order ln -> ln(1-.) -> copy: the copy is not
            # on the critical chain, but scheduled first it delays both lns
            # (and with them the DVE products) by ~3.5us.
            add_dep_helper(
                i_copy.ins, i_ln1m.ins, sync=False,
                reason="order score lns before the sum(yt) copy",
            )
            scr = scpool.tile([P, FS], F16, tag="scr")
            i_stt1 = nc.vector.scalar_tensor_tensor(
                out=scr[:], in0=yt, scalar=1.0, in1=lnp[:],
                op0=OP.mult, op1=OP.mult,
                accum_out=st[:, N_GT + 2 : N_GT + 3],
            )
            scr2 = scpool.tile([P, FS], F16, tag="scr")
            i_stt2 = nc.vector.scalar_tensor_tensor(
                out=scr2[:], in0=yt, scalar=1.0, in1=ln1m[:],
                op0=OP.mult, op1=OP.mult,
                accum_out=st[:, N_GT + 3 : N_GT + 4],
            )

            # ---------------- geometry part: 1 fused DVE op per pair-tile ---
            # Pin DVE static order [h0, h1, h2, clamp, h3, stt1, h4, stt2,
            # h5]: matches the arrival ramp, score products fill the DVE
            # while big tiles stream in, and nothing trails the last byte
            # (engine programs are static; a mis-slotted op head-of-line
            # blocks the engine).
            hs = []
            for i in range(N_GT):
                f = FGS[i]
                h = wpool.tile([P, f], F16, tag="h")
                i_h = nc.vector._custom_dve(
                    huber_op,
                    out=h[:],
                    in0=xg[i][:, 0:f],
                    in1=xg[i][:, f : 2 * f],
                    s0=0.0, s1=0.0, imm2=0.5,
                    accum_out=st[:, i : i + 1],
                )
                hs.append(i_h)
            add_dep_helper(i_clamp.ins, hs[2].ins, sync=False,
                           reason="clamp after huber 2")
            add_dep_helper(i_stt1.ins, hs[3].ins, sync=False,
                           reason="stt1 after huber 3")
            add_dep_helper(hs[4].ins, i_stt1.ins, sync=False,
                           reason="huber 4 after stt1")
            add_dep_helper(i_stt2.ins, hs[4].ins, sync=False,
                           reason="stt2 after huber 4")
            add_dep_helper(hs[5].ins, i_stt2.ins, sync=False,
                           reason="huber 5 after stt2")

            nc.sync.dma_start(out=st_d[:], in_=st[:])
    nc.finalize()
    return nc


def _get_nc():
    global _CACHED_NC
    if _CACHED_NC is None:
        _CACHED_NC = _build_nc()
    return _CACHED_NC


def _make_in_maps(Y_true_score, Y_pred_score, Y_true_geometry, Y_pred_geometry):
    FG_TOT = FG_OFF[-1]  # 16384 geometry elems per partition per tensor
    yts = np.asarray(Y_true_score, dtype=np.float32).reshape(N_CORES, P, FS)
    yps = np.asarray(Y_pred_score, dtype=np.float32).reshape(N_CORES, P, FS)
    ytg = np.asarray(Y_true_geometry, dtype=np.float32).reshape(N_CORES, P, FG_TOT)
    ypg = np.asarray(Y_pred_geometry, dtype=np.float32).reshape(N_CORES, P, FG_TOT)

    xs = np.empty((N_CORES, P, 2 * FS), dtype=np.float16)
    xs[:, :, 0:FS] = yts
    xs[:, :, FS:] = yps
    np8 = mybir.dt.np(F8)
    xgs = []
    for i in range(N_GT):
        o, f = FG_OFF[i], FGS[i]
        xg = np.empty((N_CORES, P, 2 * f), dtype=np8)
        xg[:, :, 0:f] = ytg[:, :, o : o + f]
        xg[:, :, f:] = ypg[:, :, o : o + f]
        xgs.append(xg)

    return [
        {"xs": xs[k], **{f"xg{i}": xgs[i][k] for i in range(N_GT)}}
        for k in range(N_CORES)
    ]


def _combine(results):
    """results: list of per-core dicts with st [P, NS] fp32."""
    huber_sum = 0.0
    ln1m_sum = 0.0
    yt_sum = 0.0
    t1_sum = 0.0  # sum yt*ln(yp)
    t2_sum = 0.0  # sum yt*ln(1-yp)
    for r in results:
        s = np.asarray(r["st"], dtype=np.float64)
        huber_sum += s[:, 0:N_GT].sum()
        ln1m_sum += s[:, N_GT].sum()
        yt_sum += s[:, N_GT + 1].sum()
        t1_sum += s[:, N_GT + 2].sum()
        t2_sum += s[:, N_GT + 3].sum()

    size = float(M * 1 * H * W)
    beta = 1.0 - yt_sum / size
    A = t1_sum  # sum(yt * ln yp)
    B = ln1m_sum - t2_sum  # sum((1-yt) * ln(1-yp))
    loss_score = (-beta * A - (1.0 - beta) * B) / M

    n_pix = M * H * W
    loss_geom = huber_sum / GC / n_pix  # LAMBDA_GEOMETRY = 1.0

    return np.array(loss_score + loss_geom, dtype=np.float32)


def kernel(Y_true_score, Y_pred_score, Y_true_geometry, Y_pred_geometry, **_kw):
    nc = _get_nc()
    in_maps = _make_in_maps(
        Y_true_score, Y_pred_score, Y_true_geometry, Y_pred_geometry
    )
    res = run_bass_kernel_spmd(nc, in_maps, core_ids=list(range(N_CORES)))
    return _combine(res.results)
